# revision 1
# baseline (speedup 1.0000x reference)
"""Trainium2 Bass kernel for nn_CodedNet (roll -> binary mask -> unroll -> channel sum).

Math simplification: the forward roll by -ch, the 64x64 binary mask multiply,
and the backward roll by +ch collapse to

    out[b,i,w] = sum_ch x[b,i,w,ch] * mask32[(i-ch)%32, w%32]

where mask32 = sign(w_in).reshape(32,32)  (the 64x64 mask is a 2x2 tile of it).

Strategy: pure data parallel over batch (512 -> 64 per core on 8 cores).
Each core processes 32 tiles of [128 partitions = 2 batches x 64 rows,
1984 free = 64 w x 31 ch]: DMA in, elementwise multiply by the
host-precomputed sign tensor (from w), segmented reduce over the
31-channel groups, DMA out.
"""

import sys

if "/opt/trn_rl_repo" not in sys.path:
    sys.path.insert(0, "/opt/trn_rl_repo")

import numpy as np

B, H, W, CH = 512, 64, 64, 31
N_CORES = 8
B_PER_CORE = B // N_CORES  # 64
B_PER_TILE = 2  # 2 batches x 64 rows = 128 partitions
N_TILES = B_PER_CORE // B_PER_TILE  # 32
FREE = W * CH  # 1984

TRACE = False

_nc_cache: dict = {}


def _emit_body(tc, x, m, out, variant: str):
    """One full pass over the per-core shard."""
    import concourse.mybir as mybir

    nc = tc.nc
    f32 = mybir.dt.float32

    xv = x.rearrange("(t b) i w c -> t (b i) (w c)", b=B_PER_TILE)  # [32,128,1984]
    ov = out.rearrange("(t b) i w -> t (b i) w", b=B_PER_TILE)  # [32,128,64]

    with (
        tc.tile_pool(name="mconst", bufs=1) as mpool,
        tc.tile_pool(name="work", bufs=4) as pool,
    ):
        mt = mpool.tile([128, FREE], f32)
        nc.sync.dma_start(out=mt[:], in_=m)
        for t in range(N_TILES):
            xt = pool.tile([128, FREE], f32)
            nc.sync.dma_start(out=xt[:], in_=xv[t])
            if variant == "dma":  # DMA-in only: measures HBM read bandwidth
                nc.sync.dma_start(out=ov[t], in_=xt[:, :W])
                continue
            red = pool.tile([128, W], f32)
            if variant == "v1":
                prod = pool.tile([128, FREE], f32)
                nc.vector.tensor_mul(out=prod[:], in0=xt[:], in1=mt[:])
                nc.vector.reduce_sum(
                    out=red[:],
                    in_=prod[:].rearrange("p (w c) -> p w c", c=CH),
                    axis=mybir.AxisListType.X,
                )
            elif variant == "mult_only":  # multiply, skip reduce (wrong result)
                prod = pool.tile([128, FREE], f32)
                nc.vector.tensor_mul(out=prod[:], in0=xt[:], in1=mt[:])
                nc.vector.tensor_copy(out=red[:], in_=prod[:, : W])
            elif variant == "reduce_only":  # reduce, skip multiply (wrong result)
                nc.vector.reduce_sum(
                    out=red[:],
                    in_=xt[:].rearrange("p (w c) -> p w c", c=CH),
                    axis=mybir.AxisListType.X,
                )
            elif variant == "v2":  # multiply split DVE/GPSIMD, reduce on DVE
                prod = pool.tile([128, FREE], f32)
                eng = nc.vector if t % 2 == 0 else nc.gpsimd
                eng.tensor_mul(out=prod[:], in0=xt[:], in1=mt[:])
                nc.vector.reduce_sum(
                    out=red[:],
                    in_=prod[:].rearrange("p (w c) -> p w c", c=CH),
                    axis=mybir.AxisListType.X,
                )
            else:
                raise ValueError(variant)
            nc.sync.dma_start(out=ov[t], in_=red[:])


def _emit_body_v3(tc, x, m2, out, in_place: bool, out_ring=None, bufs=4):
    """Fused tiles: 4 batches per tile ([128, 3968]), one mult + one reduce."""
    import concourse.mybir as mybir

    nc = tc.nc
    f32 = mybir.dt.float32
    bpt = 4  # batches per fused tile
    n_tiles = B_PER_CORE // bpt  # 16
    if out_ring is None:
        out_ring = nc.sync

    # [16, 128, 2, 1984]: tile t covers batches 4t..4t+3; partition=(b%2, i)
    # via (g b) with g the outer pair index inside the tile
    xv = x.rearrange("(t g b) i w c -> t (b i) g (w c)", g=2, b=2)
    ov = out.rearrange("(t g b) i w -> t (b i) g w", g=2, b=2)

    with (
        tc.tile_pool(name="mconst", bufs=1) as mpool,
        tc.tile_pool(name="work", bufs=bufs) as pool,
        tc.tile_pool(name="red", bufs=4) as rpool,
    ):
        mt = mpool.tile([128, 2 * FREE], f32)
        nc.sync.dma_start(out=mt[:], in_=m2)
        for t in range(n_tiles):
            xt = pool.tile([128, 2 * FREE], f32)
            xtv = xt[:].rearrange("p (g f) -> p g f", g=2)
            # two 1MB DMAs per fused tile
            nc.sync.dma_start(out=xtv[:, 0], in_=xv[t, :, 0])
            nc.sync.dma_start(out=xtv[:, 1], in_=xv[t, :, 1])
            if in_place:
                prodap = xt[:]
            else:
                prod = pool.tile([128, 2 * FREE], f32)
                prodap = prod[:]
            nc.vector.tensor_mul(out=prodap, in0=xt[:], in1=mt[:])
            red = rpool.tile([128, 2 * W], f32)
            nc.vector.reduce_sum(
                out=red[:].rearrange("p (g w) -> p g w", g=2),
                in_=prodap.rearrange("p (g w c) -> p g w c", g=2, c=CH),
                axis=mybir.AxisListType.X,
            )
            out_ring.dma_start(
                out=ov[t], in_=red[:].rearrange("p (g w) -> p g w", g=2)
            )


def build_nc(variant: str = "v1", reps: int = 1):
    key = (variant, reps)
    if key in _nc_cache:
        return _nc_cache[key]

    import concourse.bacc as bacc
    import concourse.mybir as mybir
    import concourse.tile as tile

    f32 = mybir.dt.float32
    nc = bacc.Bacc("TRN2", debug=False, num_devices=N_CORES)
    x = nc.dram_tensor("x", [B_PER_CORE, H, W, CH], f32, kind="ExternalInput").ap()
    m_free = 2 * FREE if variant.startswith(("v3", "v4", "v5", "v6")) else FREE
    m = nc.dram_tensor("m", [128, m_free], f32, kind="ExternalInput").ap()
    out = nc.dram_tensor("out", [B_PER_CORE, H, W], f32, kind="ExternalOutput").ap()

    with tile.TileContext(nc) as tc:
        for _ in range(reps):
            if variant == "v4":
                _emit_body_v3(tc, x, m, out, in_place=False, out_ring=nc.scalar)
            elif variant == "v5":
                _emit_body_v3(
                    tc, x, m, out, in_place=False, out_ring=nc.scalar, bufs=6
                )
            elif variant == "v6":
                _emit_body_v3(tc, x, m, out, in_place=False, bufs=6)
            elif variant.startswith(("v3", "v4", "v5")):
                _emit_body_v3(tc, x, m, out, in_place=variant == "v3ip")
            else:
                _emit_body(tc, x, m, out, variant)

    nc.compile()
    _nc_cache[key] = nc
    return nc


def host_sign_tensor(w: np.ndarray) -> np.ndarray:
    """M_rep[p, w*31+ch] = mask32[((p%64)-ch)%32, w%32], shape [128, 1984] f32."""
    mask32 = np.sign(w.astype(np.float32)).reshape(32, 32)
    i_idx = np.arange(H)
    ch_idx = np.arange(CH)
    rel = (i_idx[:, None] - ch_idx[None, :]) % 32  # [64, 31]
    w_mod = np.arange(W) % 32
    M = mask32[rel[:, None, :], w_mod[None, :, None]]  # [64, 64, 31]
    M = np.ascontiguousarray(M.reshape(H, FREE), dtype=np.float32)
    return np.tile(M, (B_PER_TILE, 1))  # [128, 1984]


def kernel(x: np.ndarray, w: np.ndarray) -> np.ndarray:
    from concourse.bass_utils import run_bass_kernel_spmd

    x = np.ascontiguousarray(np.asarray(x), dtype=np.float32)
    # v3 layout: sign tensor tiled twice along free dim ([128, 3968])
    m_rep = np.tile(host_sign_tensor(np.asarray(w)), (1, 2))

    nc = build_nc("v3", 1)
    in_maps = [
        {"x": x[c * B_PER_CORE : (c + 1) * B_PER_CORE], "m": m_rep}
        for c in range(N_CORES)
    ]
    res = run_bass_kernel_spmd(nc, in_maps, core_ids=list(range(N_CORES)), trace=TRACE)
    if TRACE and res.exec_time_ns is not None:
        kernel.last_exec_time_ns = res.exec_time_ns
    return np.concatenate([r["out"] for r in res.results], axis=0)


kernel.last_exec_time_ns = None

